# revision 3
# baseline (speedup 1.0000x reference)
"""Self-contained TRN2 Bass kernel for causal self-attention (B=2,T=2048,D=1024,H=16).

kernel(**inputs) takes the full unsharded inputs and returns the full output.
Sharding: 8 NeuronCores; core c -> batch b=c//4, head-group g=c%4 (4 heads).
Each core: projections + RoPE + causal flash attention (deferred softmax
normalization) + partial output projection; host sums 4 partials per batch.

v3 vs v2:
- RoPE merge copies (the largest DVE sink in v2) moved to SBUF->SBUF DMAs:
  rope writes split y1/y2 tiles, band DMAs build the merged head-interleaved
  kc/qc tiles off-engine.
- Score PSUM tiles are half-slab ([P, 4, 256]) so a bufs=2 pool ping-pongs
  under the ACT exp stream with the AV accumulator resident (8 banks total).
- Flash starts right after K + Q-chunk3; remaining Q chunks and V-projection
  tiles run as PE filler bursts inside the exp-bound flash (DVE staging with
  broadcast-bias adds).
- Reciprocal chain: SBUF->SBUF DMA reshape of the sums row -> DVE
  recip_approx_fast -> one DRAM bounce for the 64-partition broadcast;
  recip enqueued late to avoid DVE FIFO head-of-line blocking.
- avs/normalize path in bf16 (2x DVE); output partials fp16 (half DMA).
- PE warmup spin at t=0 opens the HAM clock gate before the first real
  matmul; startup DMAs ordered by first use.
"""

import math
from contextlib import ExitStack

import numpy as np

import concourse.bass as bass
import concourse.tile as tile
from concourse import bacc, mybir

F32 = mybir.dt.float32
BF16 = mybir.dt.bfloat16
FP16 = mybir.dt.float16

B, T, D, H, HD = 2, 2048, 1024, 16, 64
P = 128
KT = D // P            # 8 k-slabs for projections
NT = T // P            # 16 t/k tiles
QS = 512               # q-slab width for attention
HQS = 256              # half-slab (score psum tile width)
NQS = T // QS          # 4 q-slabs
HPG = 4                # heads per core


def build_nc(num_devices=8):
    nc = bacc.Bacc("TRN2", target_bir_lowering=False, debug=False,
                   num_devices=num_devices)
    ext = dict(kind="ExternalInput")
    xT = nc.dram_tensor("xT", [D, T], BF16, **ext).ap()
    wq = nc.dram_tensor("wq", [D, 2 * P], BF16, **ext).ap()
    wk = nc.dram_tensor("wk", [D, 2 * P], BF16, **ext).ap()
    wv = nc.dram_tensor("wv", [D, 2 * P], BF16, **ext).ap()
    wo = nc.dram_tensor("wo", [2 * P, D], BF16, **ext).ap()
    csc = nc.dram_tensor("csc", [P, T], BF16, **ext).ap()
    ssc = nc.dram_tensor("ssc", [P, T], BF16, **ext).ap()
    bq2 = nc.dram_tensor("bq2", [P, 2], F32, **ext).ap()
    bk2 = nc.dram_tensor("bk2", [P, 2], F32, **ext).ap()
    bvr = nc.dram_tensor("bvr", [1, 2 * P], BF16, **ext).ap()
    m0 = nc.dram_tensor("m0", [P, P], BF16, **ext).ap()
    ones_in = nc.dram_tensor("ones_in", [P, P], BF16, **ext).ap()
    out = nc.dram_tensor("out", [T, D], FP16, kind="ExternalOutput").ap()
    dbg = None
    if DEBUG_DUMP:
        dbg = nc.dram_tensor("dbg", [HD + 1, HPG, QS], BF16,
                             kind="ExternalOutput").ap()

    with tile.TileContext(nc) as tc:
        _body(tc, xT, wq, wk, wv, wo, csc, ssc, bq2, bk2, bvr, m0,
              ones_in, out, dbg)
    nc.compile()
    return nc


DEBUG_DUMP = False


def _body(tc, xT, wq, wk, wv, wo, csc, ssc, bq2, bk2, bvr, m0,
          ones_in, out, dbg=None):
    nc = tc.nc
    Exp = mybir.ActivationFunctionType.Exp
    Ident = mybir.ActivationFunctionType.Identity

    with ExitStack() as outer:
        consts = outer.enter_context(tc.tile_pool(name="consts", bufs=1))
        wpool = outer.enter_context(tc.tile_pool(name="w", bufs=1))
        xtp = outer.enter_context(tc.tile_pool(name="xt", bufs=1))
        qk = outer.enter_context(tc.tile_pool(name="qk", bufs=1))
        vp = outer.enter_context(tc.tile_pool(name="v", bufs=1))
        atp = outer.enter_context(tc.tile_pool(name="at", bufs=1))
        cscp = outer.enter_context(tc.tile_pool(name="cs", bufs=1))
        rtmp = outer.enter_context(tc.tile_pool(name="rtmp", bufs=4))

        # ---------- input DMAs, ordered by first use ----------
        w_q = wpool.tile([P, KT, 2 * P], BF16, tag="wq")
        w_k = wpool.tile([P, KT, 2 * P], BF16, tag="wk")
        wv_s = wpool.tile([P, KT, 2 * P], BF16, tag="wv")
        wkr = wk.rearrange("(ko ki) m -> ki ko m", ki=P)
        nc.sync.dma_start(w_k[:, 0, :], wkr[:, 0, :])

        xts = []
        xTr = xT.rearrange("(ko ki) t -> ki ko t", ki=P)
        for kt in range(KT):
            xc = xtp.tile([P, T], BF16, tag=f"xt{kt}")
            xts.append(xc)
        # wave 1: x + wk only (K projection's critical inputs), spread
        # over all three DMA queues so nothing else steals HBM bandwidth
        for n in range(4):
            cols = slice(n * 512, (n + 1) * 512)
            nc.sync.dma_start(xts[0][:, cols], xTr[:, 0, cols])
        bq_s = consts.tile([P, 2], F32, tag="bq")
        bk_s = consts.tile([P, 2], F32, tag="bk")
        nc.scalar.dma_start(bk_s[:], bk2)
        nc.scalar.dma_start(bq_s[:], bq2)
        nc.scalar.dma_start(xts[1][:], xTr[:, 1, :])
        nc.gpsimd.dma_start(xts[2][:], xTr[:, 2, :])
        nc.sync.dma_start(w_k[:, 1:KT, :], wkr[:, 1:KT, :])
        nc.sync.dma_start(xts[3][:], xTr[:, 3, :])
        nc.scalar.dma_start(xts[4][:], xTr[:, 4, :])
        nc.gpsimd.dma_start(xts[5][:], xTr[:, 5, :])
        nc.sync.dma_start(xts[6][:], xTr[:, 6, :])
        nc.scalar.dma_start(xts[7][:], xTr[:, 7, :])
        # wave 2: everything else, by first use
        wqr = wq.rearrange("(ko ki) m -> ki ko m", ki=P)
        nc.gpsimd.dma_start(w_q[:], wqr)
        csc_s = cscp.tile([P, T], BF16, tag="c")
        nc.scalar.dma_start(csc_s[:], csc)
        ssc_s = cscp.tile([P, T], BF16, tag="s")
        nc.scalar.dma_start(ssc_s[:], ssc)
        wvr = wv.rearrange("(ko ki) m -> ki ko m", ki=P)
        nc.gpsimd.dma_start(wv_s[:], wvr)
        bv_s = consts.tile([1, 2 * P], BF16, tag="bv")
        nc.gpsimd.dma_start(bv_s[:], bvr)
        ones_s = consts.tile([1, P], BF16, tag="ones")
        nc.gpsimd.dma_start(ones_s[:], ones_in[0:1, :])
        m0_s = consts.tile([P, P], BF16, tag="m0")
        nc.gpsimd.dma_start(m0_s[:], m0)
        wop = outer.enter_context(tc.tile_pool(name="wo", bufs=1))
        wo_s = wop.tile([P, 2, D], BF16)
        nc.gpsimd.dma_start(
            wo_s[:], wo.rearrange("(ko ki) m -> ki ko m", ki=P))

        # V sbuf store (per kt-tile, per head, HD cols + ones col for sums)
        v_s = vp.tile([P, NT, HPG, HD + 1], BF16)
        nc.vector.memset(v_s[:, :, :, HD:HD + 1], 1.0)

        # rope split outputs: y1/y2 interleaved in the free dim so one DMA
        # per head-pair builds the merged (y,d)-interleaved kc/qc tiles.
        # Contraction order only has to MATCH between k and q.
        Yk = qk.tile([P, 2, T], BF16, tag="Yk")
        Yq = qk.tile([P, 2, T], BF16, tag="Yq")
        rawk0 = qk.tile([P, T], BF16, tag="rawk0")
        rawk1 = qk.tile([P, T], BF16, tag="rawk1")
        rawq0 = qk.tile([P, T], BF16, tag="rawq0")
        rawq1 = qk.tile([P, T], BF16, tag="rawq1")
        kc = [qk.tile([P, T], BF16, tag=f"kc{j}", name=f"kc{j}")
              for j in range(2)]
        qc = [qk.tile([P, T], BF16, tag=f"qc{j}", name=f"qc{j}")
              for j in range(2)]

        at0 = atp.tile([P, T], BF16, tag="at0")
        at1 = atp.tile([P, T], BF16, tag="at1")
        at_tiles = (at0, at1)

        # warmup spin tile (zeros; results discarded)
        warm_src = consts.tile([P, 512], BF16, tag="warm")
        nc.vector.memset(warm_src[:], 0.0)

        def rope_chunk(raw0, raw1, Y, cols):
            t1 = rtmp.tile([P, 512], BF16, tag="rt", name="t1")
            nc.vector.tensor_mul(t1[:], raw0[:, cols], csc_s[:, cols])
            t2 = rtmp.tile([P, 512], BF16, tag="rt", name="t2")
            nc.vector.tensor_mul(t2[:], raw1[:, cols], ssc_s[:, cols])
            nc.vector.tensor_sub(Y[:, 0, cols], t1[:], t2[:])
            t3 = rtmp.tile([P, 512], BF16, tag="rt", name="t3")
            nc.vector.tensor_mul(t3[:], raw0[:, cols], ssc_s[:, cols])
            t4 = rtmp.tile([P, 512], BF16, tag="rt", name="t4")
            nc.vector.tensor_mul(t4[:], raw1[:, cols], csc_s[:, cols])
            nc.vector.tensor_add(Y[:, 1, cols], t3[:], t4[:])

        def merge_dma(Y, tgts, cols):
            # dst partition 2p+y <- src (p, y): per-head (d,y)-interleaved
            # contraction layout, identical for k and q.
            for j in range(2):
                nc.sync.dma_start(tgts[j][:, cols],
                                  Y[64 * j:64 * j + 64, :, cols])

        # ============ phase A ============
        with ExitStack() as pha:
            psA = pha.enter_context(
                tc.tile_pool(name="psA", bufs=8, space="PSUM"))

            warm_ps = psA.tile([P, 512], F32, tag="ps", name="warm")
            for i in range(7):
                nc.tensor.matmul(warm_ps[:], warm_src[:, 0:P], warm_src[:],
                                 start=True, stop=True)

            # K projection: psum tiles per (m, n); kt-accumulation
            pss = []
            for _i in range(8):
                pst = psA.tile([P, 512], F32, tag="ps", name=f"ps{_i}")
                pss.append(pst)
            for kt in range(KT):
                for m in range(2):
                    for n in range(4):
                        nc.tensor.matmul(
                            pss[m * 4 + n][:],
                            w_k[:, kt, m * P:(m + 1) * P],
                            xts[kt][:, n * 512:(n + 1) * 512],
                            start=(kt == 0), stop=(kt == KT - 1),
                        )
            # stage + rope + merge per chunk (ascending k for the flash)
            for n in range(4):
                cols = slice(n * 512, (n + 1) * 512)
                for m, raw in ((0, rawk0), (1, rawk1)):
                    nc.scalar.activation(
                        out=raw[:, cols], in_=pss[m * 4 + n][:],
                        func=Ident, bias=bk_s[:, m:m + 1], scale=1.0)
                rope_chunk(rawk0, rawk1, Yk, cols)
                merge_dma(Yk, kc, cols)

            # Q chunk 3 (flash qs=3 reads q cols 1536+ first)
            qps = [psA.tile([P, 512], F32, tag="ps", name=f"qps{m}")
                   for m in range(2)]
            for kt in range(KT):
                for m in range(2):
                    nc.tensor.matmul(
                        qps[m][:],
                        w_q[:, kt, m * P:(m + 1) * P],
                        xts[kt][:, 1536:2048],
                        start=(kt == 0), stop=(kt == KT - 1),
                    )
            cols3 = slice(1536, 2048)
            for m, raw in ((0, rawq0), (1, rawq1)):
                nc.scalar.activation(
                    out=raw[:, cols3], in_=qps[m][:],
                    func=Ident, bias=bq_s[:, m:m + 1], scale=1.0)
            rope_chunk(rawq0, rawq1, Yq, cols3)
            merge_dma(Yq, qc, cols3)

            # V tiles 0-3 (needed by flash kt=0..3), staged on ACT
            for t in range(4):
                vss = psA.tile([P, 2 * P], F32, tag="ps", name=f"vss{t}")
                for kt in range(KT):
                    nc.tensor.matmul(
                        vss[:], xts[kt][:, t * P:(t + 1) * P],
                        wv_s[:, kt, :],
                        start=(kt == 0), stop=False)
                nc.tensor.matmul(vss[:], ones_s[:], bv_s[:],
                                 start=False, stop=True)
                nc.scalar.activation(
                    out=v_s[:, t, :, 0:HD],
                    in_=vss.rearrange("p (h d) -> p h d", h=HPG),
                    func=Ident, scale=1.0)

        # ================= flash phase =================
        with ExitStack() as phb:
            expp = phb.enter_context(tc.tile_pool(name="exp", bufs=6))
            avsp = phb.enter_context(tc.tile_pool(name="avs", bufs=2))
            rrp = phb.enter_context(tc.tile_pool(name="rr", bufs=2))
            s16p = phb.enter_context(tc.tile_pool(name="s16", bufs=2))
            ps_sc = phb.enter_context(
                tc.tile_pool(name="pssc", bufs=2, space="PSUM"))
            ps_av = phb.enter_context(
                tc.tile_pool(name="psav", bufs=1, space="PSUM"))
            drp = phb.enter_context(
                tc.tile_pool(name="dr", bufs=2, space="DRAM"))
            obp = phb.enter_context(tc.tile_pool(name="obp", bufs=3))

            post = {}       # qs -> (avs, s16b)
            rdy = {}        # qs -> (avs, rr)
            av_tiles = {}

            def emit_vtile(t):
                vss = ps_sc.tile([P, 2 * P], F32, tag="sc", name=f"v{t}")
                for kt in range(KT):
                    nc.tensor.matmul(
                        vss[:], xts[kt][:, t * P:(t + 1) * P],
                        wv_s[:, kt, :], start=(kt == 0), stop=False)
                nc.tensor.matmul(vss[:], ones_s[:], bv_s[:],
                                 start=False, stop=True)
                nc.vector.tensor_copy(
                    v_s[:, t, :, 0:HD],
                    vss.rearrange("p (h d) -> p h d", h=HPG))

            def emit_qchunk_m(ch, m):
                cols = slice(ch * 512, (ch + 1) * 512)
                qp = ps_sc.tile([P, 512], F32, tag="sc", name=f"q{ch}{m}")
                for kt in range(KT):
                    nc.tensor.matmul(
                        qp[:], w_q[:, kt, m * P:(m + 1) * P],
                        xts[kt][:, cols], start=(kt == 0), stop=(kt == KT - 1))
                raw = rawq0 if m == 0 else rawq1
                nc.vector.tensor_add(
                    raw[:, cols], qp[:],
                    bq_s[:, m:m + 1].broadcast_to([P, 512]))

            def emit_qrope(ch):
                cols = slice(ch * 512, (ch + 1) * 512)
                rope_chunk(rawq0, rawq1, Yq, cols)
                merge_dma(Yq, qc, cols)

            def emit_rchain_head(qs):
                avs = avsp.tile([HD + 1, HPG, QS], BF16, tag="avs",
                                name="avs")
                nc.vector.tensor_copy(avs[:], av_tiles[qs][:])
                s16b = s16p.tile([16, P], BF16, tag="s16b", name="s16b")
                nc.sync.dma_start(
                    s16b[:],
                    avs[HD:HD + 1, :, :].rearrange("o h q -> o (h q)"))
                post[qs] = (avs, s16b)

            def emit_rchain_tail(qs):
                avs, s16b = post.pop(qs)
                s16f = s16p.tile([16, P], F32, tag="s16f", name="s16f")
                nc.vector.tensor_copy(s16f[:], s16b[:])
                r16f = s16p.tile([16, P], F32, tag="r16f", name="r16f")
                nc.vector.reciprocal_approx_fast(out=r16f[:], in_=s16f[:])
                r16b = s16p.tile([16, P], BF16, tag="r16b", name="r16b")
                nc.vector.tensor_copy(r16b[:], r16f[:])
                d_r = drp.tile([HPG * QS], BF16, tag="dr", name="d_r")
                nc.sync.dma_start(d_r.rearrange("(p c) -> p c", p=16),
                                  r16b[:])
                rr = rrp.tile([HD, HPG, QS], BF16, tag="rr", name="rr")
                nc.sync.dma_start(
                    rr[:], d_r[None, :].broadcast_to([HD, HPG * QS])
                    .rearrange("p (h q) -> p h q", h=HPG))
                rdy[qs] = (avs, rr)

            def emit_normalize(qs, fine=False):
                # avs/rr are slot-indexed (slot = 2*(h%2) + h//2)
                avs, rr = rdy.pop(qs)
                if not fine:
                    for tg in range(2):
                        for i in range(2):
                            h = 2 * tg + i
                            sl = 2 * (h % 2) + h // 2
                            nc.vector.tensor_mul(
                                at_tiles[tg][i * HD:(i + 1) * HD,
                                             qs * QS:(qs + 1) * QS],
                                avs[0:HD, sl, :], rr[:, sl, :])
                else:
                    for qt in range(4 * qs, 4 * qs + 4):
                        c0 = qt * P - qs * QS
                        for tg in range(2):
                            for i in range(2):
                                h = 2 * tg + i
                                sl = 2 * (h % 2) + h // 2
                                nc.vector.tensor_mul(
                                    at_tiles[tg][i * HD:(i + 1) * HD,
                                                 qt * P:(qt + 1) * P],
                                    avs[0:HD, sl, c0:c0 + P],
                                    rr[:, sl, c0:c0 + P])
                        emit_outproj_qt(qt, ring=nc.scalar)

            def emit_outproj_qt(qt, ring=None):
                ps = ps_sc.tile([P, 2, 512], F32, tag="sc", name="ops")
                ob = obp.tile([P, D], FP16, tag="ob", name="ob")
                for ktg in range(2):
                    for nb in range(2):
                        nc.tensor.matmul(
                            ps[:, nb, :],
                            at_tiles[ktg][:, qt * P:(qt + 1) * P],
                            wo_s[:, ktg, nb * 512:(nb + 1) * 512],
                            start=(ktg == 0), stop=(ktg == 1))
                for nb in range(2):
                    nc.vector.tensor_copy(
                        out=ob[:, nb * 512:(nb + 1) * 512],
                        in_=ps[:, nb, :])
                if ring is None:
                    ring = nc.gpsimd
                ring.dma_start(out[qt * P:(qt + 1) * P, :], ob[:])


            fillers = {}

            def add_filler(qidx, kt, fn):
                fillers.setdefault((qidx, kt), []).append(fn)

            # qs=3 (qidx 0): V tile t=kt+3 at kt 1..12, Q chunk2 bursts
            # at kt 13/14 (+rope), Q chunk1 m0 at kt 15.
            for kt in range(1, 13):
                add_filler(0, kt, (lambda t=kt + 3: emit_vtile(t)))
            add_filler(0, 13, lambda: emit_qchunk_m(2, 0))
            add_filler(0, 14, lambda: emit_qchunk_m(2, 1))
            add_filler(0, 14, lambda: emit_qrope(2))
            add_filler(0, 15, lambda: emit_qchunk_m(1, 0))
            def finish_prev(qs):
                emit_rchain_tail(qs)
                emit_normalize(qs)
                for qt in range(4 * qs, 4 * qs + 4):
                    emit_outproj_qt(qt)

            # qs=2 (qidx 1)
            add_filler(1, 0, lambda: emit_qchunk_m(1, 1))
            add_filler(1, 0, lambda: emit_qrope(1))
            add_filler(1, 2, lambda: emit_qchunk_m(0, 0))
            add_filler(1, 3, lambda: emit_qchunk_m(0, 1))
            add_filler(1, 3, lambda: emit_qrope(0))
            add_filler(1, 10, lambda: finish_prev(3))
            # qs=1 (qidx 2)
            add_filler(2, 6, lambda: finish_prev(2))
            # qs=0 (qidx 3): recip chain late enough that the diag masks
            # of kt 0-1 are already past the DVE queue head
            add_filler(3, 1, lambda: emit_rchain_tail(1))

            QORDER = [3, 2, 1, 0]

            def emit_av(av, ex, qs, kt, hh, hoff, n_kt):
                c0, c1 = hh * HQS, (hh + 1) * HQS
                stop_kt = (4 * qs + 1) if hh == 0 else (n_kt - 1)
                for h in range(HPG):
                    slot = 2 * (h % 2) + h // 2
                    nc.tensor.matmul(
                        av[:, slot, c0 + hoff:c1],
                        v_s[:, kt, h, :],
                        ex[:, slot * HQS + hoff:(slot + 1) * HQS],
                        start=(kt == 0 and hh == 0),
                        stop=(kt == stop_kt),
                        skip_group_check=True)

            for qidx, qs in enumerate(QORDER):
                av = ps_av.tile([HD + 1, HPG, QS], F32, tag="av", name="av")
                av_tiles[qs] = av
                n_kt = 4 * qs + 4
                pend = []
                for kt in range(n_kt):
                    for fn in fillers.get((qidx, kt), ()):
                        fn()
                    qoff = max(0, kt * P - qs * QS)
                    diag = kt * P >= qs * QS
                    for hh in range(2):
                        c0, c1 = hh * HQS, (hh + 1) * HQS
                        if qoff >= c1:
                            continue
                        hoff = max(0, qoff - c0)
                        q0 = qs * QS + c0 + hoff
                        qext = HQS - hoff
                        # slot = 2*i + pair so the two concurrently-packed
                        # matmuls (i=0 rows 0-63, i=1 rows 64-127) write
                        # DIFFERENT psum banks (hw requirement). 2D tiles so
                        # the common-case exp AP is one flat span (~0.87us
                        # vs 1.11us for the 4-subdim strided form).
                        sc = ps_sc.tile([P, HPG * HQS], F32, tag="sc",
                                        name="sc")
                        for pair in range(2):
                            for i in range(2):
                                sl = 2 * i + pair
                                nc.tensor.matmul(
                                    sc[:, sl * HQS + hoff:(sl + 1) * HQS],
                                    kc[pair][64 * i:64 * i + 64,
                                             kt * P:(kt + 1) * P],
                                    qc[pair][64 * i:64 * i + 64,
                                             q0:q0 + qext],
                                    start=True, stop=True,
                                    tile_position=(64 * i, 0))
                        ex = expp.tile([P, HPG * HQS], BF16, tag="ex",
                                       name="ex")
                        if hoff == 0:
                            nc.scalar.activation(
                                out=ex[:], in_=sc[:], func=Exp, scale=1.0)
                        else:
                            for sl in range(HPG):
                                nc.scalar.activation(
                                    out=ex[:, sl * HQS + hoff:
                                           (sl + 1) * HQS],
                                    in_=sc[:, sl * HQS + hoff:
                                           (sl + 1) * HQS],
                                    func=Exp, scale=1.0)
                        if diag and c0 <= qoff:
                            exv = ex.rearrange("p (h q) -> p h q", h=HPG)
                            nc.vector.tensor_mul(
                                exv[:, :, hoff:hoff + P],
                                exv[:, :, hoff:hoff + P],
                                m0_s[:, None, :].broadcast_to([P, HPG, P]))
                        pend.append((ex, kt, hh, hoff))
                    # AV runs one kt behind scores so filler bursts between
                    # iterations never starve the ACT exp stream.
                    while pend and pend[0][1] < kt:
                        e, k2, h2, o2 = pend.pop(0)
                        emit_av(av, e, qs, k2, h2, o2, n_kt)
                while pend:
                    e, k2, h2, o2 = pend.pop(0)
                    emit_av(av, e, qs, k2, h2, o2, n_kt)
                emit_rchain_head(qs)

            emit_normalize(1)
            emit_outproj_qt(4, ring=nc.scalar)
            for qt in (5, 6, 7):
                emit_outproj_qt(qt)
            emit_rchain_tail(0)
            emit_normalize(0, fine=True)


# ---------------- host-side prep ----------------

def _perm(g):
    # m-major: m=0 -> x1 channels (even rope dims), m=1 -> x2 (odd)
    perm = []
    for m in range(2):
        for h in range(HPG):
            for i in range(32):
                perm.append(256 * g + 64 * h + 2 * i + m)
    return np.array(perm)


def host_inputs(inputs, c):
    b, g = c // 4, c % 4
    x, cos, sin = inputs["x"], inputs["cos"], inputs["sin"]
    Wq, bq, Wk, bk = inputs["Wq"], inputs["bq"], inputs["Wk"], inputs["bk"]
    Wv, bv, Wo = inputs["Wv"], inputs["bv"], inputs["Wo"]
    perm = _perm(g)
    s = math.sqrt(1.0 / math.sqrt(HD))
    cosT = np.ascontiguousarray(cos[0, 0].T) * s    # [32, T]
    sinT = np.ascontiguousarray(sin[0, 0].T) * s
    f32 = np.float32
    import ml_dtypes
    bf16 = ml_dtypes.bfloat16
    return {
        "xT": np.ascontiguousarray(x[b].T).astype(bf16),
        "wq": np.ascontiguousarray(Wq[perm, :].T).astype(bf16),
        "wk": np.ascontiguousarray(Wk[perm, :].T).astype(bf16),
        "wv": np.ascontiguousarray(Wv[256 * g:256 * (g + 1), :].T).astype(bf16),
        "wo": np.ascontiguousarray(Wo[:, 256 * g:256 * (g + 1)].T).astype(bf16),
        "csc": np.ascontiguousarray(np.tile(cosT, (4, 1))).astype(bf16),
        "ssc": np.ascontiguousarray(np.tile(sinT, (4, 1))).astype(bf16),
        "bq2": np.ascontiguousarray(bq[perm].reshape(2, P).T).astype(f32),
        "bk2": np.ascontiguousarray(bk[perm].reshape(2, P).T).astype(f32),
        "bvr": np.ascontiguousarray(
            bv[256 * g:256 * (g + 1)].reshape(1, 2 * P)).astype(bf16),
        "m0": np.ascontiguousarray(
            (np.arange(P)[None, :] >= np.arange(P)[:, None])).astype(bf16),
        "ones_in": np.ones((P, P), bf16),
    }


def host_gather(results, bo):
    out = np.zeros((B, T, D), np.float32)
    for c in range(8):
        out[c // 4] += results[c]["out"].astype(np.float32)
    out += bo[None, None, :]
    return out


_NC_CACHE = {}


def _get_nc():
    if "nc" not in _NC_CACHE:
        _NC_CACHE["nc"] = build_nc(num_devices=8)
    return _NC_CACHE["nc"]


def kernel(**inputs):
    inputs = {k: np.asarray(v) for k, v in inputs.items()}
    nc = _get_nc()
    from concourse.bass_utils import run_bass_kernel_spmd
    in_maps = [host_inputs(inputs, c) for c in range(8)]
    res = run_bass_kernel_spmd(nc, in_maps, core_ids=list(range(8)))
    return host_gather(res.results, inputs["bo"].astype(np.float32))


# revision 4
# speedup vs baseline: 1.0781x; 1.0781x over previous
"""Self-contained TRN2 Bass kernel for causal self-attention (B=2,T=2048,D=1024,H=16).

kernel(**inputs) takes the full unsharded inputs and returns the full output.
Sharding: 8 NeuronCores; core c -> batch b=c//4, head-group g=c%4 (4 heads).
Each core: projections + RoPE + causal flash attention (deferred softmax
normalization) + partial output projection; host sums 4 partials per batch.

v3 vs v2:
- RoPE merge copies (the largest DVE sink in v2) moved to SBUF->SBUF DMAs:
  rope writes split y1/y2 tiles, band DMAs build the merged head-interleaved
  kc/qc tiles off-engine.
- Score PSUM tiles are half-slab ([P, 4, 256]) so a bufs=2 pool ping-pongs
  under the ACT exp stream with the AV accumulator resident (8 banks total).
- Flash starts right after K + Q-chunk3; remaining Q chunks and V-projection
  tiles run as PE filler bursts inside the exp-bound flash (DVE staging with
  broadcast-bias adds).
- Reciprocal chain: SBUF->SBUF DMA reshape of the sums row -> DVE
  recip_approx_fast -> one DRAM bounce for the 64-partition broadcast;
  recip enqueued late to avoid DVE FIFO head-of-line blocking.
- avs/normalize path in bf16 (2x DVE); output partials fp16 (half DMA).
- PE warmup spin at t=0 opens the HAM clock gate before the first real
  matmul; startup DMAs ordered by first use.
"""

import math
from contextlib import ExitStack

import numpy as np

import concourse.bass as bass
import concourse.tile as tile
from concourse import bacc, mybir

F32 = mybir.dt.float32
BF16 = mybir.dt.bfloat16
FP16 = mybir.dt.float16

B, T, D, H, HD = 2, 2048, 1024, 16, 64
P = 128
KT = D // P            # 8 k-slabs for projections
NT = T // P            # 16 t/k tiles
QS = 512               # q-slab width for attention
HQS = 256              # half-slab (score psum tile width)
NQS = T // QS          # 4 q-slabs
HPG = 4                # heads per core


def build_nc(num_devices=8):
    nc = bacc.Bacc("TRN2", target_bir_lowering=False, debug=False,
                   num_devices=num_devices)
    ext = dict(kind="ExternalInput")
    xT = nc.dram_tensor("xT", [D, T], BF16, **ext).ap()
    wq = nc.dram_tensor("wq", [D, 2 * P], BF16, **ext).ap()
    wk = nc.dram_tensor("wk", [D, 2 * P], BF16, **ext).ap()
    wv = nc.dram_tensor("wv", [D, 2 * P], BF16, **ext).ap()
    wo = nc.dram_tensor("wo", [2 * P, D], BF16, **ext).ap()
    csc = nc.dram_tensor("csc", [P, T], BF16, **ext).ap()
    ssc = nc.dram_tensor("ssc", [P, T], BF16, **ext).ap()
    bq2 = nc.dram_tensor("bq2", [P, 2], F32, **ext).ap()
    bk2 = nc.dram_tensor("bk2", [P, 2], F32, **ext).ap()
    bvr = nc.dram_tensor("bvr", [1, 2 * P], BF16, **ext).ap()
    m0 = nc.dram_tensor("m0", [P, P], BF16, **ext).ap()
    ones_in = nc.dram_tensor("ones_in", [P, P], BF16, **ext).ap()
    out = nc.dram_tensor("out", [T, D], FP16, kind="ExternalOutput").ap()
    dbg = None
    if DEBUG_DUMP:
        dbg = nc.dram_tensor("dbg", [HD + 1, HPG, QS], BF16,
                             kind="ExternalOutput").ap()

    with tile.TileContext(nc) as tc:
        _body(tc, xT, wq, wk, wv, wo, csc, ssc, bq2, bk2, bvr, m0,
              ones_in, out, dbg)
    nc.compile()
    return nc


DEBUG_DUMP = False


def _body(tc, xT, wq, wk, wv, wo, csc, ssc, bq2, bk2, bvr, m0,
          ones_in, out, dbg=None):
    nc = tc.nc
    Exp = mybir.ActivationFunctionType.Exp
    Ident = mybir.ActivationFunctionType.Identity

    with ExitStack() as outer:
        consts = outer.enter_context(tc.tile_pool(name="consts", bufs=1))
        wpool = outer.enter_context(tc.tile_pool(name="w", bufs=1))
        xtp = outer.enter_context(tc.tile_pool(name="xt", bufs=1))
        qk = outer.enter_context(tc.tile_pool(name="qk", bufs=1))
        vp = outer.enter_context(tc.tile_pool(name="v", bufs=1))
        atp = outer.enter_context(tc.tile_pool(name="at", bufs=1))
        cscp = outer.enter_context(tc.tile_pool(name="cs", bufs=1))
        rtmp = outer.enter_context(tc.tile_pool(name="rtmp", bufs=4))

        # ---------- input DMAs, ordered by first use ----------
        w_q = wpool.tile([P, KT, 2 * P], BF16, tag="wq")
        w_k = wpool.tile([P, KT, 2 * P], BF16, tag="wk")
        wv_s = wpool.tile([P, KT, 2 * P], BF16, tag="wv")
        wkr = wk.rearrange("(ko ki) m -> ki ko m", ki=P)
        nc.sync.dma_start(w_k[:, 0, :], wkr[:, 0, :])

        xts = []
        xTr = xT.rearrange("(ko ki) t -> ki ko t", ki=P)
        for kt in range(KT):
            xc = xtp.tile([P, T], BF16, tag=f"xt{kt}")
            xts.append(xc)
        # wave 1: x + wk only (K projection's critical inputs), spread
        # over all three DMA queues so nothing else steals HBM bandwidth
        for n in range(4):
            cols = slice(n * 512, (n + 1) * 512)
            nc.sync.dma_start(xts[0][:, cols], xTr[:, 0, cols])
        bq_s = consts.tile([P, 2], F32, tag="bq")
        bk_s = consts.tile([P, 2], F32, tag="bk")
        nc.scalar.dma_start(bk_s[:], bk2)
        nc.scalar.dma_start(bq_s[:], bq2)
        nc.scalar.dma_start(xts[1][:], xTr[:, 1, :])
        nc.gpsimd.dma_start(xts[2][:], xTr[:, 2, :])
        nc.sync.dma_start(w_k[:, 1:KT, :], wkr[:, 1:KT, :])
        nc.sync.dma_start(xts[3][:], xTr[:, 3, :])
        nc.scalar.dma_start(xts[4][:], xTr[:, 4, :])
        nc.gpsimd.dma_start(xts[5][:], xTr[:, 5, :])
        nc.sync.dma_start(xts[6][:], xTr[:, 6, :])
        nc.scalar.dma_start(xts[7][:], xTr[:, 7, :])
        # wave 2: everything else, by first use
        wqr = wq.rearrange("(ko ki) m -> ki ko m", ki=P)
        nc.gpsimd.dma_start(w_q[:], wqr)
        csc_s = cscp.tile([P, T], BF16, tag="c")
        nc.scalar.dma_start(csc_s[:], csc)
        ssc_s = cscp.tile([P, T], BF16, tag="s")
        nc.scalar.dma_start(ssc_s[:], ssc)
        wvr = wv.rearrange("(ko ki) m -> ki ko m", ki=P)
        nc.gpsimd.dma_start(wv_s[:], wvr)
        bv_s = consts.tile([1, 2 * P], BF16, tag="bv")
        nc.gpsimd.dma_start(bv_s[:], bvr)
        ones_s = consts.tile([1, P], BF16, tag="ones")
        nc.gpsimd.dma_start(ones_s[:], ones_in[0:1, :])
        m0_s = consts.tile([P, P], BF16, tag="m0")
        nc.gpsimd.dma_start(m0_s[:], m0)
        wop = outer.enter_context(tc.tile_pool(name="wo", bufs=1))
        wo_s = wop.tile([P, 2, D], BF16)
        nc.gpsimd.dma_start(
            wo_s[:], wo.rearrange("(ko ki) m -> ki ko m", ki=P))

        # V sbuf store (per kt-tile, per head, HD cols + ones col for sums)
        v_s = vp.tile([P, NT, HPG, HD + 1], BF16)
        nc.vector.memset(v_s[:, :, :, HD:HD + 1], 1.0)

        # rope split outputs: y1/y2 interleaved in the free dim so one DMA
        # per head-pair builds the merged (y,d)-interleaved kc/qc tiles.
        # Contraction order only has to MATCH between k and q.
        Yk = qk.tile([P, 2, T], BF16, tag="Yk")
        Yq = qk.tile([P, 2, T], BF16, tag="Yq")
        rawk0 = qk.tile([P, T], BF16, tag="rawk0")
        rawk1 = qk.tile([P, T], BF16, tag="rawk1")
        rawq0 = qk.tile([P, T], BF16, tag="rawq0")
        rawq1 = qk.tile([P, T], BF16, tag="rawq1")
        kc = [qk.tile([P, T], BF16, tag=f"kc{j}", name=f"kc{j}")
              for j in range(2)]
        qc = [qk.tile([P, T], BF16, tag=f"qc{j}", name=f"qc{j}")
              for j in range(2)]

        at0 = atp.tile([P, T], BF16, tag="at0")
        at1 = atp.tile([P, T], BF16, tag="at1")
        at_tiles = (at0, at1)

        # warmup spin tile (zeros; results discarded)
        warm_src = consts.tile([P, 512], BF16, tag="warm")
        nc.vector.memset(warm_src[:], 0.0)

        def rope_chunk(raw0, raw1, Y, cols):
            t1 = rtmp.tile([P, 512], BF16, tag="rt", name="t1")
            nc.vector.tensor_mul(t1[:], raw0[:, cols], csc_s[:, cols])
            t2 = rtmp.tile([P, 512], BF16, tag="rt", name="t2")
            nc.vector.tensor_mul(t2[:], raw1[:, cols], ssc_s[:, cols])
            nc.vector.tensor_sub(Y[:, 0, cols], t1[:], t2[:])
            t3 = rtmp.tile([P, 512], BF16, tag="rt", name="t3")
            nc.vector.tensor_mul(t3[:], raw0[:, cols], ssc_s[:, cols])
            t4 = rtmp.tile([P, 512], BF16, tag="rt", name="t4")
            nc.vector.tensor_mul(t4[:], raw1[:, cols], csc_s[:, cols])
            nc.vector.tensor_add(Y[:, 1, cols], t3[:], t4[:])

        def merge_dma(Y, tgts, cols):
            # dst partition 2p+y <- src (p, y): per-head (d,y)-interleaved
            # contraction layout, identical for k and q.
            for j in range(2):
                nc.sync.dma_start(tgts[j][:, cols],
                                  Y[64 * j:64 * j + 64, :, cols])

        # ============ phase A ============
        with ExitStack() as pha:
            psA = pha.enter_context(
                tc.tile_pool(name="psA", bufs=8, space="PSUM"))

            warm_ps = psA.tile([P, 512], F32, tag="ps", name="warm")
            for i in range(7):
                nc.tensor.matmul(warm_ps[:], warm_src[:, 0:P], warm_src[:],
                                 start=True, stop=True)

            # K projection: psum tiles per (m, n); kt-accumulation
            pss = []
            for _i in range(8):
                pst = psA.tile([P, 512], F32, tag="ps", name=f"ps{_i}")
                pss.append(pst)
            for kt in range(KT):
                for m in range(2):
                    for n in range(4):
                        nc.tensor.matmul(
                            pss[m * 4 + n][:],
                            w_k[:, kt, m * P:(m + 1) * P],
                            xts[kt][:, n * 512:(n + 1) * 512],
                            start=(kt == 0), stop=(kt == KT - 1),
                        )
            # stage + rope + merge per chunk (ascending k for the flash)
            for n in range(4):
                cols = slice(n * 512, (n + 1) * 512)
                for m, raw in ((0, rawk0), (1, rawk1)):
                    nc.scalar.activation(
                        out=raw[:, cols], in_=pss[m * 4 + n][:],
                        func=Ident, bias=bk_s[:, m:m + 1], scale=1.0)
                rope_chunk(rawk0, rawk1, Yk, cols)
                merge_dma(Yk, kc, cols)

            # Q chunk 3 (flash qs=3 reads q cols 1536+ first)
            qps = [psA.tile([P, 512], F32, tag="ps", name=f"qps{m}")
                   for m in range(2)]
            for kt in range(KT):
                for m in range(2):
                    nc.tensor.matmul(
                        qps[m][:],
                        w_q[:, kt, m * P:(m + 1) * P],
                        xts[kt][:, 1536:2048],
                        start=(kt == 0), stop=(kt == KT - 1),
                    )
            cols3 = slice(1536, 2048)
            for m, raw in ((0, rawq0), (1, rawq1)):
                nc.scalar.activation(
                    out=raw[:, cols3], in_=qps[m][:],
                    func=Ident, bias=bq_s[:, m:m + 1], scale=1.0)
            rope_chunk(rawq0, rawq1, Yq, cols3)
            merge_dma(Yq, qc, cols3)

            # V tiles 0-3 (needed by flash kt=0..3), staged on ACT
            for t in range(4):
                vss = psA.tile([P, 2 * P], F32, tag="ps", name=f"vss{t}")
                for kt in range(KT):
                    nc.tensor.matmul(
                        vss[:], xts[kt][:, t * P:(t + 1) * P],
                        wv_s[:, kt, :],
                        start=(kt == 0), stop=False)
                nc.tensor.matmul(vss[:], ones_s[:], bv_s[:],
                                 start=False, stop=True)
                nc.scalar.activation(
                    out=v_s[:, t, :, 0:HD],
                    in_=vss.rearrange("p (h d) -> p h d", h=HPG),
                    func=Ident, scale=1.0)

        # ================= flash phase =================
        with ExitStack() as phb:
            expp = phb.enter_context(tc.tile_pool(name="exp", bufs=6))
            avsp = phb.enter_context(tc.tile_pool(name="avs", bufs=2))
            rrp = phb.enter_context(tc.tile_pool(name="rr", bufs=2))
            s16p = phb.enter_context(tc.tile_pool(name="s16", bufs=2))
            ps_sc = phb.enter_context(
                tc.tile_pool(name="pssc", bufs=2, space="PSUM"))
            ps_av = phb.enter_context(
                tc.tile_pool(name="psav", bufs=1, space="PSUM"))
            drp = phb.enter_context(
                tc.tile_pool(name="dr", bufs=2, space="DRAM"))
            obp = phb.enter_context(tc.tile_pool(name="obp", bufs=3))

            post = {}       # qs -> (avs, s16b)
            rdy = {}        # qs -> (avs, rr)
            av_tiles = {}

            def emit_vtile(t):
                vss = ps_sc.tile([P, 2 * P], F32, tag="sc", name=f"v{t}")
                for kt in range(KT):
                    nc.tensor.matmul(
                        vss[:], xts[kt][:, t * P:(t + 1) * P],
                        wv_s[:, kt, :], start=(kt == 0), stop=False)
                nc.tensor.matmul(vss[:], ones_s[:], bv_s[:],
                                 start=False, stop=True)
                nc.vector.tensor_copy(
                    v_s[:, t, :, 0:HD],
                    vss.rearrange("p (h d) -> p h d", h=HPG))

            def emit_qchunk_m(ch, m):
                cols = slice(ch * 512, (ch + 1) * 512)
                qp = ps_sc.tile([P, 512], F32, tag="sc", name=f"q{ch}{m}")
                for kt in range(KT):
                    nc.tensor.matmul(
                        qp[:], w_q[:, kt, m * P:(m + 1) * P],
                        xts[kt][:, cols], start=(kt == 0), stop=(kt == KT - 1))
                raw = rawq0 if m == 0 else rawq1
                nc.vector.tensor_add(
                    raw[:, cols], qp[:],
                    bq_s[:, m:m + 1].broadcast_to([P, 512]))

            def emit_qrope(ch):
                cols = slice(ch * 512, (ch + 1) * 512)
                rope_chunk(rawq0, rawq1, Yq, cols)
                merge_dma(Yq, qc, cols)

            def emit_rchain_head(qs):
                avs = avsp.tile([HD + 1, HPG, QS], BF16, tag="avs",
                                name="avs")
                nc.vector.tensor_copy(avs[:], av_tiles[qs][:])
                s16b = s16p.tile([16, P], BF16, tag="s16b", name="s16b")
                nc.sync.dma_start(
                    s16b[:],
                    avs[HD:HD + 1, :, :].rearrange("o h q -> o (h q)"))
                post[qs] = (avs, s16b)

            def emit_rchain_tail(qs):
                avs, s16b = post.pop(qs)
                s16f = s16p.tile([16, P], F32, tag="s16f", name="s16f")
                nc.vector.tensor_copy(s16f[:], s16b[:])
                r16f = s16p.tile([16, P], F32, tag="r16f", name="r16f")
                nc.vector.reciprocal_approx_fast(out=r16f[:], in_=s16f[:])
                r16b = s16p.tile([16, P], BF16, tag="r16b", name="r16b")
                nc.vector.tensor_copy(r16b[:], r16f[:])
                d_r = drp.tile([HPG * QS], BF16, tag="dr", name="d_r")
                nc.sync.dma_start(d_r.rearrange("(p c) -> p c", p=16),
                                  r16b[:])
                rr = rrp.tile([HD, HPG, QS], BF16, tag="rr", name="rr")
                nc.sync.dma_start(
                    rr[:], d_r[None, :].broadcast_to([HD, HPG * QS])
                    .rearrange("p (h q) -> p h q", h=HPG))
                rdy[qs] = (avs, rr)

            def emit_normalize(qs, fine=False):
                # avs/rr are slot-indexed (slot = 2*(h%2) + h//2)
                avs, rr = rdy.pop(qs)
                if not fine:
                    for tg in range(2):
                        for i in range(2):
                            h = 2 * tg + i
                            sl = 2 * (h % 2) + h // 2
                            nc.vector.tensor_mul(
                                at_tiles[tg][i * HD:(i + 1) * HD,
                                             qs * QS:(qs + 1) * QS],
                                avs[0:HD, sl, :], rr[:, sl, :])
                else:
                    for qt in range(4 * qs, 4 * qs + 4):
                        c0 = qt * P - qs * QS
                        for tg in range(2):
                            for i in range(2):
                                h = 2 * tg + i
                                sl = 2 * (h % 2) + h // 2
                                nc.vector.tensor_mul(
                                    at_tiles[tg][i * HD:(i + 1) * HD,
                                                 qt * P:(qt + 1) * P],
                                    avs[0:HD, sl, c0:c0 + P],
                                    rr[:, sl, c0:c0 + P])
                        emit_outproj_qt(qt, ring=nc.scalar)

            def emit_outproj_qt(qt, ring=None):
                ps = ps_sc.tile([P, 2, 512], F32, tag="sc", name="ops")
                ob = obp.tile([P, D], FP16, tag="ob", name="ob")
                for ktg in range(2):
                    for nb in range(2):
                        nc.tensor.matmul(
                            ps[:, nb, :],
                            at_tiles[ktg][:, qt * P:(qt + 1) * P],
                            wo_s[:, ktg, nb * 512:(nb + 1) * 512],
                            start=(ktg == 0), stop=(ktg == 1))
                for nb in range(2):
                    nc.vector.tensor_copy(
                        out=ob[:, nb * 512:(nb + 1) * 512],
                        in_=ps[:, nb, :])
                if ring is None:
                    ring = nc.gpsimd
                ring.dma_start(out[qt * P:(qt + 1) * P, :], ob[:])


            fillers = {}

            def add_filler(qidx, kt, fn):
                fillers.setdefault((qidx, kt), []).append(fn)

            # qs=3 (qidx 0): V tile t=kt+3 at kt 1..12, Q chunk2 bursts
            # at kt 13/14 (+rope), Q chunk1 m0 at kt 15.
            for kt in range(1, 13):
                add_filler(0, kt, (lambda t=kt + 3: emit_vtile(t)))
            add_filler(0, 13, lambda: emit_qchunk_m(2, 0))
            add_filler(0, 14, lambda: emit_qchunk_m(2, 1))
            add_filler(0, 14, lambda: emit_qrope(2))
            add_filler(0, 15, lambda: emit_qchunk_m(1, 0))
            def finish_prev(qs):
                emit_rchain_tail(qs)
                emit_normalize(qs)
                for qt in range(4 * qs, 4 * qs + 4):
                    emit_outproj_qt(qt)

            # qs=2 (qidx 1)
            add_filler(1, 0, lambda: emit_qchunk_m(1, 1))
            add_filler(1, 0, lambda: emit_qrope(1))
            add_filler(1, 2, lambda: emit_qchunk_m(0, 0))
            add_filler(1, 3, lambda: emit_qchunk_m(0, 1))
            add_filler(1, 3, lambda: emit_qrope(0))
            add_filler(1, 10, lambda: finish_prev(3))
            # qs=1 (qidx 2)
            add_filler(2, 6, lambda: finish_prev(2))
            # qs=0 (qidx 3)
            add_filler(3, 0, lambda: emit_rchain_tail(1))
            add_filler(3, 0, lambda: emit_normalize(1))

            QORDER = [3, 2, 1, 0]

            def emit_av(av, ex, qs, kt, hh, hoff, n_kt):
                c0, c1 = hh * HQS, (hh + 1) * HQS
                stop_kt = (4 * qs + 1) if hh == 0 else (n_kt - 1)
                for h in range(HPG):
                    slot = 2 * (h % 2) + h // 2
                    nc.tensor.matmul(
                        av[:, slot, c0 + hoff:c1],
                        v_s[:, kt, h, :],
                        ex[:, slot, hoff:HQS],
                        start=(kt == 0 and hh == 0),
                        stop=(kt == stop_kt),
                        skip_group_check=True)

            for qidx, qs in enumerate(QORDER):
                av = ps_av.tile([HD + 1, HPG, QS], F32, tag="av", name="av")
                av_tiles[qs] = av
                n_kt = 4 * qs + 4
                pend = []
                for kt in range(n_kt):
                    for fn in fillers.get((qidx, kt), ()):
                        fn()
                    qoff = max(0, kt * P - qs * QS)
                    diag = kt * P >= qs * QS
                    for hh in range(2):
                        c0, c1 = hh * HQS, (hh + 1) * HQS
                        if qoff >= c1:
                            continue
                        hoff = max(0, qoff - c0)
                        q0 = qs * QS + c0 + hoff
                        qext = HQS - hoff
                        # slot = 2*i + pair so the two concurrently-packed
                        # matmuls (i=0 rows 0-63, i=1 rows 64-127) write
                        # DIFFERENT psum banks (hw requirement).
                        sc = ps_sc.tile([P, HPG, HQS], F32, tag="sc",
                                        name="sc")
                        for pair in range(2):
                            for i in range(2):
                                nc.tensor.matmul(
                                    sc[:, 2 * i + pair, hoff:HQS],
                                    kc[pair][64 * i:64 * i + 64,
                                             kt * P:(kt + 1) * P],
                                    qc[pair][64 * i:64 * i + 64,
                                             q0:q0 + qext],
                                    start=True, stop=True,
                                    tile_position=(64 * i, 0))
                        ex = expp.tile([P, HPG, HQS], BF16, tag="ex",
                                       name="ex")
                        nc.scalar.activation(
                            out=ex[:, :, hoff:HQS],
                            in_=sc[:, :, hoff:HQS],
                            func=Exp, scale=1.0)
                        if diag and c0 <= qoff:
                            nc.vector.tensor_mul(
                                ex[:, :, hoff:hoff + P],
                                ex[:, :, hoff:hoff + P],
                                m0_s[:, None, :].broadcast_to([P, HPG, P]))
                        pend.append((ex, kt, hh, hoff))
                    # AV runs one kt behind scores so filler bursts between
                    # iterations never starve the ACT exp stream.
                    while pend and pend[0][1] < kt:
                        e, k2, h2, o2 = pend.pop(0)
                        emit_av(av, e, qs, k2, h2, o2, n_kt)
                while pend:
                    e, k2, h2, o2 = pend.pop(0)
                    emit_av(av, e, qs, k2, h2, o2, n_kt)
                emit_rchain_head(qs)

            emit_outproj_qt(4, ring=nc.scalar)
            for qt in (5, 6, 7):
                emit_outproj_qt(qt)
            emit_rchain_tail(0)
            emit_normalize(0, fine=True)


# ---------------- host-side prep ----------------

def _perm(g):
    # m-major: m=0 -> x1 channels (even rope dims), m=1 -> x2 (odd)
    perm = []
    for m in range(2):
        for h in range(HPG):
            for i in range(32):
                perm.append(256 * g + 64 * h + 2 * i + m)
    return np.array(perm)


def host_inputs(inputs, c):
    b, g = c // 4, c % 4
    x, cos, sin = inputs["x"], inputs["cos"], inputs["sin"]
    Wq, bq, Wk, bk = inputs["Wq"], inputs["bq"], inputs["Wk"], inputs["bk"]
    Wv, bv, Wo = inputs["Wv"], inputs["bv"], inputs["Wo"]
    perm = _perm(g)
    s = math.sqrt(1.0 / math.sqrt(HD))
    cosT = np.ascontiguousarray(cos[0, 0].T) * s    # [32, T]
    sinT = np.ascontiguousarray(sin[0, 0].T) * s
    f32 = np.float32
    import ml_dtypes
    bf16 = ml_dtypes.bfloat16
    return {
        "xT": np.ascontiguousarray(x[b].T).astype(bf16),
        "wq": np.ascontiguousarray(Wq[perm, :].T).astype(bf16),
        "wk": np.ascontiguousarray(Wk[perm, :].T).astype(bf16),
        "wv": np.ascontiguousarray(Wv[256 * g:256 * (g + 1), :].T).astype(bf16),
        "wo": np.ascontiguousarray(Wo[:, 256 * g:256 * (g + 1)].T).astype(bf16),
        "csc": np.ascontiguousarray(np.tile(cosT, (4, 1))).astype(bf16),
        "ssc": np.ascontiguousarray(np.tile(sinT, (4, 1))).astype(bf16),
        "bq2": np.ascontiguousarray(bq[perm].reshape(2, P).T).astype(f32),
        "bk2": np.ascontiguousarray(bk[perm].reshape(2, P).T).astype(f32),
        "bvr": np.ascontiguousarray(
            bv[256 * g:256 * (g + 1)].reshape(1, 2 * P)).astype(bf16),
        "m0": np.ascontiguousarray(
            (np.arange(P)[None, :] >= np.arange(P)[:, None])).astype(bf16),
        "ones_in": np.ones((P, P), bf16),
    }


def host_gather(results, bo):
    out = np.zeros((B, T, D), np.float32)
    for c in range(8):
        out[c // 4] += results[c]["out"].astype(np.float32)
    out += bo[None, None, :]
    return out


_NC_CACHE = {}


def _get_nc():
    if "nc" not in _NC_CACHE:
        _NC_CACHE["nc"] = build_nc(num_devices=8)
    return _NC_CACHE["nc"]


def kernel(**inputs):
    inputs = {k: np.asarray(v) for k, v in inputs.items()}
    nc = _get_nc()
    from concourse.bass_utils import run_bass_kernel_spmd
    in_maps = [host_inputs(inputs, c) for c in range(8)]
    res = run_bass_kernel_spmd(nc, in_maps, core_ids=list(range(8)))
    return host_gather(res.results, inputs["bo"].astype(np.float32))


# revision 5
# speedup vs baseline: 1.1065x; 1.0264x over previous
"""Self-contained TRN2 Bass kernel for causal self-attention (B=2,T=2048,D=1024,H=16).

kernel(**inputs) takes the full unsharded inputs and returns the full output.
Sharding: 8 NeuronCores; core c -> batch b=c//4, head-group g=c%4 (4 heads).
Each core: projections + RoPE + causal flash attention (deferred softmax
normalization) + partial output projection; host sums 4 partials per batch.

v3 vs v2:
- RoPE merge copies (the largest DVE sink in v2) moved to SBUF->SBUF DMAs:
  rope writes split y1/y2 tiles, band DMAs build the merged head-interleaved
  kc/qc tiles off-engine.
- Score PSUM tiles are half-slab ([P, 4, 256]) so a bufs=2 pool ping-pongs
  under the ACT exp stream with the AV accumulator resident (8 banks total).
- Flash starts right after K + Q-chunk3; remaining Q chunks and V-projection
  tiles run as PE filler bursts inside the exp-bound flash (DVE staging with
  broadcast-bias adds).
- Reciprocal chain: SBUF->SBUF DMA reshape of the sums row -> DVE
  recip_approx_fast -> one DRAM bounce for the 64-partition broadcast;
  recip enqueued late to avoid DVE FIFO head-of-line blocking.
- avs/normalize path in bf16 (2x DVE); output partials fp16 (half DMA).
- PE warmup spin at t=0 opens the HAM clock gate before the first real
  matmul; startup DMAs ordered by first use.
"""

import math
from contextlib import ExitStack

import numpy as np

import concourse.bass as bass
import concourse.tile as tile
from concourse import bacc, mybir

F32 = mybir.dt.float32
BF16 = mybir.dt.bfloat16
FP16 = mybir.dt.float16

B, T, D, H, HD = 2, 2048, 1024, 16, 64
P = 128
KT = D // P            # 8 k-slabs for projections
NT = T // P            # 16 t/k tiles
QS = 512               # q-slab width for attention
HQS = 256              # half-slab (score psum tile width)
NQS = T // QS          # 4 q-slabs
HPG = 4                # heads per core


def build_nc(num_devices=8):
    nc = bacc.Bacc("TRN2", target_bir_lowering=False, debug=False,
                   num_devices=num_devices)
    ext = dict(kind="ExternalInput")
    xT = nc.dram_tensor("xT", [D, T], BF16, **ext).ap()
    wq = nc.dram_tensor("wq", [D, 2 * P], BF16, **ext).ap()
    wk = nc.dram_tensor("wk", [D, 2 * P], BF16, **ext).ap()
    wv = nc.dram_tensor("wv", [D, 2 * P], BF16, **ext).ap()
    wo = nc.dram_tensor("wo", [2 * P, D], BF16, **ext).ap()
    csc = nc.dram_tensor("csc", [P, T], BF16, **ext).ap()
    ssc = nc.dram_tensor("ssc", [P, T], BF16, **ext).ap()
    bq2 = nc.dram_tensor("bq2", [P, 2], F32, **ext).ap()
    bk2 = nc.dram_tensor("bk2", [P, 2], F32, **ext).ap()
    bvr = nc.dram_tensor("bvr", [1, 2 * P], BF16, **ext).ap()
    m0 = nc.dram_tensor("m0", [P, P], BF16, **ext).ap()
    ones_in = nc.dram_tensor("ones_in", [P, P], BF16, **ext).ap()
    out = nc.dram_tensor("out", [T, D], FP16, kind="ExternalOutput").ap()
    dbg = None
    if DEBUG_DUMP:
        dbg = nc.dram_tensor("dbg", [HD + 1, HPG, QS], BF16,
                             kind="ExternalOutput").ap()

    with tile.TileContext(nc) as tc:
        _body(tc, xT, wq, wk, wv, wo, csc, ssc, bq2, bk2, bvr, m0,
              ones_in, out, dbg)
    nc.compile()
    return nc


DEBUG_DUMP = False


def _body(tc, xT, wq, wk, wv, wo, csc, ssc, bq2, bk2, bvr, m0,
          ones_in, out, dbg=None):
    nc = tc.nc
    Exp = mybir.ActivationFunctionType.Exp
    Ident = mybir.ActivationFunctionType.Identity

    with ExitStack() as outer:
        consts = outer.enter_context(tc.tile_pool(name="consts", bufs=1))
        wpool = outer.enter_context(tc.tile_pool(name="w", bufs=1))
        xtp = outer.enter_context(tc.tile_pool(name="xt", bufs=1))
        qk = outer.enter_context(tc.tile_pool(name="qk", bufs=1))
        vp = outer.enter_context(tc.tile_pool(name="v", bufs=1))
        atp = outer.enter_context(tc.tile_pool(name="at", bufs=1))
        cscp = outer.enter_context(tc.tile_pool(name="cs", bufs=1))
        rtmp = outer.enter_context(tc.tile_pool(name="rtmp", bufs=4))

        # ---------- input DMAs, ordered by first use ----------
        w_q = wpool.tile([P, KT, 2 * P], BF16, tag="wq")
        w_k = wpool.tile([P, KT, 2 * P], BF16, tag="wk")
        wv_s = wpool.tile([P, KT, 2 * P], BF16, tag="wv")
        wkr = wk.rearrange("(ko ki) m -> ki ko m", ki=P)
        nc.sync.dma_start(w_k[:, 0, :], wkr[:, 0, :])

        xts = []
        xTr = xT.rearrange("(ko ki) t -> ki ko t", ki=P)
        for kt in range(KT):
            xc = xtp.tile([P, T], BF16, tag=f"xt{kt}")
            xts.append(xc)
        # wave 1: x + wk only (K projection's critical inputs), spread
        # over all three DMA queues so nothing else steals HBM bandwidth
        for n in range(4):
            cols = slice(n * 512, (n + 1) * 512)
            nc.sync.dma_start(xts[0][:, cols], xTr[:, 0, cols])
        bq_s = consts.tile([P, 2], F32, tag="bq")
        bk_s = consts.tile([P, 2], F32, tag="bk")
        nc.scalar.dma_start(bk_s[:], bk2)
        nc.scalar.dma_start(bq_s[:], bq2)
        nc.scalar.dma_start(xts[1][:], xTr[:, 1, :])
        nc.gpsimd.dma_start(xts[2][:], xTr[:, 2, :])
        nc.sync.dma_start(w_k[:, 1:KT, :], wkr[:, 1:KT, :])
        nc.sync.dma_start(xts[3][:], xTr[:, 3, :])
        nc.scalar.dma_start(xts[4][:], xTr[:, 4, :])
        nc.gpsimd.dma_start(xts[5][:], xTr[:, 5, :])
        nc.sync.dma_start(xts[6][:], xTr[:, 6, :])
        nc.scalar.dma_start(xts[7][:], xTr[:, 7, :])
        # wave 2: everything else, by first use
        wqr = wq.rearrange("(ko ki) m -> ki ko m", ki=P)
        nc.gpsimd.dma_start(w_q[:], wqr)
        csc_s = cscp.tile([P, T], BF16, tag="c")
        nc.scalar.dma_start(csc_s[:], csc)
        ssc_s = cscp.tile([P, T], BF16, tag="s")
        nc.scalar.dma_start(ssc_s[:], ssc)
        wvr = wv.rearrange("(ko ki) m -> ki ko m", ki=P)
        nc.gpsimd.dma_start(wv_s[:], wvr)
        bv_s = consts.tile([1, 2 * P], BF16, tag="bv")
        nc.gpsimd.dma_start(bv_s[:], bvr)
        ones_s = consts.tile([1, P], BF16, tag="ones")
        nc.gpsimd.dma_start(ones_s[:], ones_in[0:1, :])
        m0_s = consts.tile([P, P], BF16, tag="m0")
        nc.gpsimd.dma_start(m0_s[:], m0)
        wop = outer.enter_context(tc.tile_pool(name="wo", bufs=1))
        wo_s = wop.tile([P, 2, D], BF16)
        nc.gpsimd.dma_start(
            wo_s[:], wo.rearrange("(ko ki) m -> ki ko m", ki=P))

        # V sbuf store (per kt-tile, per head, HD cols + ones col for sums)
        v_s = vp.tile([P, NT, HPG, HD + 1], BF16)
        nc.vector.memset(v_s[:, :, :, HD:HD + 1], 1.0)

        # rope split outputs: y1/y2 interleaved in the free dim so one DMA
        # per head-pair builds the merged (y,d)-interleaved kc/qc tiles.
        # Contraction order only has to MATCH between k and q.
        Yk = qk.tile([P, 2, T], BF16, tag="Yk")
        Yq = qk.tile([P, 2, T], BF16, tag="Yq")
        rawk0 = qk.tile([P, T], BF16, tag="rawk0")
        rawk1 = qk.tile([P, T], BF16, tag="rawk1")
        rawq0 = qk.tile([P, T], BF16, tag="rawq0")
        rawq1 = qk.tile([P, T], BF16, tag="rawq1")
        kc = [qk.tile([P, T], BF16, tag=f"kc{j}", name=f"kc{j}")
              for j in range(2)]
        qc = [qk.tile([P, T], BF16, tag=f"qc{j}", name=f"qc{j}")
              for j in range(2)]

        at0 = atp.tile([P, T], BF16, tag="at0")
        at1 = atp.tile([P, T], BF16, tag="at1")
        at_tiles = (at0, at1)

        # warmup spin tile (zeros; results discarded)
        warm_src = consts.tile([P, 512], BF16, tag="warm")
        nc.vector.memset(warm_src[:], 0.0)

        def rope_chunk(raw0, raw1, Y, cols):
            t1 = rtmp.tile([P, 512], BF16, tag="rt", name="t1")
            nc.vector.tensor_mul(t1[:], raw0[:, cols], csc_s[:, cols])
            t2 = rtmp.tile([P, 512], BF16, tag="rt", name="t2")
            nc.vector.tensor_mul(t2[:], raw1[:, cols], ssc_s[:, cols])
            nc.vector.tensor_sub(Y[:, 0, cols], t1[:], t2[:])
            t3 = rtmp.tile([P, 512], BF16, tag="rt", name="t3")
            nc.vector.tensor_mul(t3[:], raw0[:, cols], ssc_s[:, cols])
            t4 = rtmp.tile([P, 512], BF16, tag="rt", name="t4")
            nc.vector.tensor_mul(t4[:], raw1[:, cols], csc_s[:, cols])
            nc.vector.tensor_add(Y[:, 1, cols], t3[:], t4[:])

        def merge_dma(Y, tgts, cols):
            # dst partition 2p+y <- src (p, y): per-head (d,y)-interleaved
            # contraction layout, identical for k and q.
            for j in range(2):
                nc.sync.dma_start(tgts[j][:, cols],
                                  Y[64 * j:64 * j + 64, :, cols])

        # ============ phase A ============
        with ExitStack() as pha:
            psA = pha.enter_context(
                tc.tile_pool(name="psA", bufs=8, space="PSUM"))

            warm_ps = psA.tile([P, 512], F32, tag="ps", name="warm")
            for i in range(4):
                nc.tensor.matmul(warm_ps[:], warm_src[:, 0:P], warm_src[:],
                                 start=True, stop=True)

            # K projection: psum tiles per (m, n); kt-accumulation
            pss = []
            for _i in range(8):
                pst = psA.tile([P, 512], F32, tag="ps", name=f"ps{_i}")
                pss.append(pst)
            for kt in range(KT):
                for m in range(2):
                    for n in range(4):
                        nc.tensor.matmul(
                            pss[m * 4 + n][:],
                            w_k[:, kt, m * P:(m + 1) * P],
                            xts[kt][:, n * 512:(n + 1) * 512],
                            start=(kt == 0), stop=(kt == KT - 1),
                        )
            # stage + rope + merge per chunk (ascending k for the flash)
            for n in range(4):
                cols = slice(n * 512, (n + 1) * 512)
                for m, raw in ((0, rawk0), (1, rawk1)):
                    nc.scalar.activation(
                        out=raw[:, cols], in_=pss[m * 4 + n][:],
                        func=Ident, bias=bk_s[:, m:m + 1], scale=1.0)
                rope_chunk(rawk0, rawk1, Yk, cols)
                merge_dma(Yk, kc, cols)

            # Q chunk 3 (flash qs=3 reads q cols 1536+ first)
            qps = [psA.tile([P, 512], F32, tag="ps", name=f"qps{m}")
                   for m in range(2)]
            for kt in range(KT):
                for m in range(2):
                    nc.tensor.matmul(
                        qps[m][:],
                        w_q[:, kt, m * P:(m + 1) * P],
                        xts[kt][:, 1536:2048],
                        start=(kt == 0), stop=(kt == KT - 1),
                    )
            cols3 = slice(1536, 2048)
            for m, raw in ((0, rawq0), (1, rawq1)):
                nc.scalar.activation(
                    out=raw[:, cols3], in_=qps[m][:],
                    func=Ident, bias=bq_s[:, m:m + 1], scale=1.0)
            rope_chunk(rawq0, rawq1, Yq, cols3)
            merge_dma(Yq, qc, cols3)

            # V tiles 0-3 (needed by flash kt=0..3), staged on ACT
            for t in range(4):
                vss = psA.tile([P, 2 * P], F32, tag="ps", name=f"vss{t}")
                for kt in range(KT):
                    nc.tensor.matmul(
                        vss[:], xts[kt][:, t * P:(t + 1) * P],
                        wv_s[:, kt, :],
                        start=(kt == 0), stop=False)
                nc.tensor.matmul(vss[:], ones_s[:], bv_s[:],
                                 start=False, stop=True)
                nc.scalar.activation(
                    out=v_s[:, t, :, 0:HD],
                    in_=vss.rearrange("p (h d) -> p h d", h=HPG),
                    func=Ident, scale=1.0)

        # ================= flash phase =================
        with ExitStack() as phb:
            expp = phb.enter_context(tc.tile_pool(name="exp", bufs=6))
            avsp = phb.enter_context(tc.tile_pool(name="avs", bufs=2))
            rrp = phb.enter_context(tc.tile_pool(name="rr", bufs=2))
            s16p = phb.enter_context(tc.tile_pool(name="s16", bufs=2))
            ps_sc = phb.enter_context(
                tc.tile_pool(name="pssc", bufs=2, space="PSUM"))
            ps_av = phb.enter_context(
                tc.tile_pool(name="psav", bufs=1, space="PSUM"))
            drp = phb.enter_context(
                tc.tile_pool(name="dr", bufs=2, space="DRAM"))
            obp = phb.enter_context(tc.tile_pool(name="obp", bufs=4))

            post = {}       # qs -> (avs, s16b)
            rdy = {}        # qs -> (avs, rr)
            av_tiles = {}

            def emit_vtile(t):
                vss = ps_sc.tile([P, 2 * P], F32, tag="sc", name=f"v{t}")
                for kt in range(KT):
                    nc.tensor.matmul(
                        vss[:], xts[kt][:, t * P:(t + 1) * P],
                        wv_s[:, kt, :], start=(kt == 0), stop=False)
                nc.tensor.matmul(vss[:], ones_s[:], bv_s[:],
                                 start=False, stop=True)
                nc.vector.tensor_copy(
                    v_s[:, t, :, 0:HD],
                    vss.rearrange("p (h d) -> p h d", h=HPG))

            def emit_qchunk_m(ch, m):
                cols = slice(ch * 512, (ch + 1) * 512)
                qp = ps_sc.tile([P, 512], F32, tag="sc", name=f"q{ch}{m}")
                for kt in range(KT):
                    nc.tensor.matmul(
                        qp[:], w_q[:, kt, m * P:(m + 1) * P],
                        xts[kt][:, cols], start=(kt == 0), stop=(kt == KT - 1))
                raw = rawq0 if m == 0 else rawq1
                nc.vector.tensor_add(
                    raw[:, cols], qp[:],
                    bq_s[:, m:m + 1].broadcast_to([P, 512]))

            def emit_qrope(ch):
                cols = slice(ch * 512, (ch + 1) * 512)
                rope_chunk(rawq0, rawq1, Yq, cols)
                merge_dma(Yq, qc, cols)

            def emit_rchain_head(qs):
                avs = avsp.tile([HD + 1, HPG, QS], BF16, tag="avs",
                                name="avs")
                nc.vector.tensor_copy(avs[:], av_tiles[qs][:])
                s16b = s16p.tile([16, P], BF16, tag="s16b", name="s16b")
                nc.sync.dma_start(
                    s16b[:],
                    avs[HD:HD + 1, :, :].rearrange("o h q -> o (h q)"))
                post[qs] = (avs, s16b)

            def emit_rchain_tail(qs):
                avs, s16b = post.pop(qs)
                s16f = s16p.tile([16, P], F32, tag="s16f", name="s16f")
                nc.vector.tensor_copy(s16f[:], s16b[:])
                r16f = s16p.tile([16, P], F32, tag="r16f", name="r16f")
                nc.vector.reciprocal_approx_fast(out=r16f[:], in_=s16f[:])
                r16b = s16p.tile([16, P], BF16, tag="r16b", name="r16b")
                nc.vector.tensor_copy(r16b[:], r16f[:])
                d_r = drp.tile([HPG * QS], BF16, tag="dr", name="d_r")
                nc.sync.dma_start(d_r.rearrange("(p c) -> p c", p=16),
                                  r16b[:])
                rr = rrp.tile([HD, HPG, QS], BF16, tag="rr", name="rr")
                nc.sync.dma_start(
                    rr[:], d_r[None, :].broadcast_to([HD, HPG * QS])
                    .rearrange("p (h q) -> p h q", h=HPG))
                rdy[qs] = (avs, rr)

            def emit_normalize(qs, fine=False):
                # avs/rr are slot-indexed (slot = 2*(h%2) + h//2)
                avs, rr = rdy.pop(qs)
                if not fine:
                    for tg in range(2):
                        for i in range(2):
                            h = 2 * tg + i
                            sl = 2 * (h % 2) + h // 2
                            nc.vector.tensor_mul(
                                at_tiles[tg][i * HD:(i + 1) * HD,
                                             qs * QS:(qs + 1) * QS],
                                avs[0:HD, sl, :], rr[:, sl, :])
                else:
                    for qt in range(4 * qs, 4 * qs + 4):
                        c0 = qt * P - qs * QS
                        for tg in range(2):
                            for i in range(2):
                                h = 2 * tg + i
                                sl = 2 * (h % 2) + h // 2
                                nc.vector.tensor_mul(
                                    at_tiles[tg][i * HD:(i + 1) * HD,
                                                 qt * P:(qt + 1) * P],
                                    avs[0:HD, sl, c0:c0 + P],
                                    rr[:, sl, c0:c0 + P])
                        emit_outproj_qt(qt, ring=nc.scalar)

            def emit_outproj_qt(qt, ring=None):
                ps = ps_sc.tile([P, 2, 512], F32, tag="sc", name="ops")
                ob = obp.tile([P, D], FP16, tag="ob", name="ob")
                for ktg in range(2):
                    for nb in range(2):
                        nc.tensor.matmul(
                            ps[:, nb, :],
                            at_tiles[ktg][:, qt * P:(qt + 1) * P],
                            wo_s[:, ktg, nb * 512:(nb + 1) * 512],
                            start=(ktg == 0), stop=(ktg == 1))
                nc.vector.tensor_copy(
                    out=ob.rearrange("p (a b) -> p a b", a=2),
                    in_=ps[:])
                if ring is None:
                    ring = nc.gpsimd
                ring.dma_start(out[qt * P:(qt + 1) * P, :], ob[:])


            fillers = {}

            def add_filler(qidx, kt, fn):
                fillers.setdefault((qidx, kt), []).append(fn)

            # qs=3 (qidx 0): V tile t=kt+3 at kt 1..12, Q chunk2 bursts
            # at kt 13/14 (+rope), Q chunk1 m0 at kt 15.
            for kt in range(1, 13):
                add_filler(0, kt, (lambda t=kt + 3: emit_vtile(t)))
            add_filler(0, 13, lambda: emit_qchunk_m(2, 0))
            add_filler(0, 14, lambda: emit_qchunk_m(2, 1))
            add_filler(0, 14, lambda: emit_qrope(2))
            add_filler(0, 15, lambda: emit_qchunk_m(1, 0))
            def finish_prev(qs):
                emit_rchain_tail(qs)
                emit_normalize(qs)
                for qt in range(4 * qs, 4 * qs + 4):
                    emit_outproj_qt(qt)

            # qs=2 (qidx 1)
            add_filler(1, 0, lambda: emit_qchunk_m(1, 1))
            add_filler(1, 0, lambda: emit_qrope(1))
            add_filler(1, 2, lambda: emit_qchunk_m(0, 0))
            add_filler(1, 3, lambda: emit_qchunk_m(0, 1))
            add_filler(1, 3, lambda: emit_qrope(0))
            add_filler(1, 10, lambda: finish_prev(3))
            # qs=1 (qidx 2)
            add_filler(2, 6, lambda: finish_prev(2))
            # qs=0 (qidx 3): recip chain late enough that the diag masks
            # of kt 0-1 are already past the DVE queue head
            add_filler(3, 1, lambda: emit_rchain_tail(1))

            QORDER = [3, 2, 1, 0]

            def emit_av(av, ex, qs, kt, hh, hoff, n_kt):
                c0, c1 = hh * HQS, (hh + 1) * HQS
                stop_kt = (4 * qs + 1) if hh == 0 else (n_kt - 1)
                for h in range(HPG):
                    slot = 2 * (h % 2) + h // 2
                    nc.tensor.matmul(
                        av[:, slot, c0 + hoff:c1],
                        v_s[:, kt, h, :],
                        ex[:, slot, hoff:HQS],
                        start=(kt == 0 and hh == 0),
                        stop=(kt == stop_kt),
                        skip_group_check=True)

            for qidx, qs in enumerate(QORDER):
                av = ps_av.tile([HD + 1, HPG, QS], F32, tag="av", name="av")
                av_tiles[qs] = av
                n_kt = 4 * qs + 4
                pend = []
                for kt in range(n_kt):
                    for fn in fillers.get((qidx, kt), ()):
                        fn()
                    qoff = max(0, kt * P - qs * QS)
                    diag = kt * P >= qs * QS
                    for hh in range(2):
                        c0, c1 = hh * HQS, (hh + 1) * HQS
                        if qoff >= c1:
                            continue
                        hoff = max(0, qoff - c0)
                        q0 = qs * QS + c0 + hoff
                        qext = HQS - hoff
                        # slot = 2*i + pair so the two concurrently-packed
                        # matmuls (i=0 rows 0-63, i=1 rows 64-127) write
                        # DIFFERENT psum banks (hw requirement).
                        sc = ps_sc.tile([P, HPG, HQS], F32, tag="sc",
                                        name="sc")
                        for pair in range(2):
                            for i in range(2):
                                nc.tensor.matmul(
                                    sc[:, 2 * i + pair, hoff:HQS],
                                    kc[pair][64 * i:64 * i + 64,
                                             kt * P:(kt + 1) * P],
                                    qc[pair][64 * i:64 * i + 64,
                                             q0:q0 + qext],
                                    start=True, stop=True,
                                    tile_position=(64 * i, 0))
                        ex = expp.tile([P, HPG, HQS], BF16, tag="ex",
                                       name="ex")
                        nc.scalar.activation(
                            out=ex[:, :, hoff:HQS],
                            in_=sc[:, :, hoff:HQS],
                            func=Exp, scale=1.0)
                        if diag and c0 <= qoff:
                            nc.vector.tensor_mul(
                                ex[:, :, hoff:hoff + P],
                                ex[:, :, hoff:hoff + P],
                                m0_s[:, None, :].broadcast_to([P, HPG, P]))
                        pend.append((ex, kt, hh, hoff))
                    # AV runs one kt behind scores so filler bursts between
                    # iterations never starve the ACT exp stream.
                    while pend and pend[0][1] < kt:
                        e, k2, h2, o2 = pend.pop(0)
                        emit_av(av, e, qs, k2, h2, o2, n_kt)
                while pend:
                    e, k2, h2, o2 = pend.pop(0)
                    emit_av(av, e, qs, k2, h2, o2, n_kt)
                emit_rchain_head(qs)

            emit_normalize(1)
            emit_outproj_qt(4, ring=nc.scalar)
            for qt in (5, 6, 7):
                emit_outproj_qt(qt)
            emit_rchain_tail(0)
            emit_normalize(0, fine=True)


# ---------------- host-side prep ----------------

def _perm(g):
    # m-major: m=0 -> x1 channels (even rope dims), m=1 -> x2 (odd)
    perm = []
    for m in range(2):
        for h in range(HPG):
            for i in range(32):
                perm.append(256 * g + 64 * h + 2 * i + m)
    return np.array(perm)


def host_inputs(inputs, c):
    b, g = c // 4, c % 4
    x, cos, sin = inputs["x"], inputs["cos"], inputs["sin"]
    Wq, bq, Wk, bk = inputs["Wq"], inputs["bq"], inputs["Wk"], inputs["bk"]
    Wv, bv, Wo = inputs["Wv"], inputs["bv"], inputs["Wo"]
    perm = _perm(g)
    s = math.sqrt(1.0 / math.sqrt(HD))
    cosT = np.ascontiguousarray(cos[0, 0].T) * s    # [32, T]
    sinT = np.ascontiguousarray(sin[0, 0].T) * s
    f32 = np.float32
    import ml_dtypes
    bf16 = ml_dtypes.bfloat16
    return {
        "xT": np.ascontiguousarray(x[b].T).astype(bf16),
        "wq": np.ascontiguousarray(Wq[perm, :].T).astype(bf16),
        "wk": np.ascontiguousarray(Wk[perm, :].T).astype(bf16),
        "wv": np.ascontiguousarray(Wv[256 * g:256 * (g + 1), :].T).astype(bf16),
        "wo": np.ascontiguousarray(Wo[:, 256 * g:256 * (g + 1)].T).astype(bf16),
        "csc": np.ascontiguousarray(np.tile(cosT, (4, 1))).astype(bf16),
        "ssc": np.ascontiguousarray(np.tile(sinT, (4, 1))).astype(bf16),
        "bq2": np.ascontiguousarray(bq[perm].reshape(2, P).T).astype(f32),
        "bk2": np.ascontiguousarray(bk[perm].reshape(2, P).T).astype(f32),
        "bvr": np.ascontiguousarray(
            bv[256 * g:256 * (g + 1)].reshape(1, 2 * P)).astype(bf16),
        "m0": np.ascontiguousarray(
            (np.arange(P)[None, :] >= np.arange(P)[:, None])).astype(bf16),
        "ones_in": np.ones((P, P), bf16),
    }


def host_gather(results, bo):
    out = np.zeros((B, T, D), np.float32)
    for c in range(8):
        out[c // 4] += results[c]["out"].astype(np.float32)
    out += bo[None, None, :]
    return out


_NC_CACHE = {}


def _get_nc():
    if "nc" not in _NC_CACHE:
        _NC_CACHE["nc"] = build_nc(num_devices=8)
    return _NC_CACHE["nc"]


def kernel(**inputs):
    inputs = {k: np.asarray(v) for k, v in inputs.items()}
    nc = _get_nc()
    from concourse.bass_utils import run_bass_kernel_spmd
    in_maps = [host_inputs(inputs, c) for c in range(8)]
    res = run_bass_kernel_spmd(nc, in_maps, core_ids=list(range(8)))
    return host_gather(res.results, inputs["bo"].astype(np.float32))
